# revision 1
# baseline (speedup 1.0000x reference)
"""Trainium2 Bass kernel for nn_Block_5583457485503 (mini transformer block).

Reference math (B=8192, T=32, C=128, H=4, D=32):
    q,k,v = per-head projections of x;  att = softmax(mask(q k^T / sqrt(D)))
    x = x + concat_h(att_h @ v_h);  x = x + relu(x@w1+b1)@w2 + b2

Sharding: data-parallel over batch across 8 cores (1024 seqs / core).
Weights replicated. Inside a core, tokens are processed in groups of
512 (16 seqs); within a group, 4 subtiles of 128 tokens (4 seqs each:
partition p = 32*b4 + s for quad-member b4, token-in-seq s).

Structure (per group):
 - x is loaded bf16 via a casting SWDGE DMA; X^T via PE identity matmuls
   (all-bf16 transposes: fp32 matmuls cost 4 cycles/row on trn2).
 - scores_h = x A_h x^T with A_h = wq_h wk_h^T * D^-0.5 precomputed on
   host, so every score contraction is 128-deep; the causal mask is
   accumulated into PSUM by one maskI^T @ maskR matmul per half.
 - The attention mid-section runs in two k-halves, each with its own
   PSUM score bank, so exp/spread/phase-2 of half A overlaps the score
   matmuls of half B.
 - An appended ones-column on V yields the softmax denominator from the
   same matmuls that compute att @ v.
 - Residuals are folded into PE work: Z^T = X^T + Tn^T by accumulating
   transposes; the FFN2 bank is seeded with Z via a ZT @ I matmul.
 - Copies are spread over DVE/ACT/GpSimd; the exp(S) block-diagonal
   spread runs on the otherwise-idle GpSimd engine.
"""

import os
import sys

import numpy as np

sys.path.insert(0, "/opt/trn_rl_repo")

NUM_EMB = 128
CONTEXT = 32
NUM_HEADS = 4
HEAD = 32
BATCH = 8192
N_CORES = 8
SEQ_PER_CORE = BATCH // N_CORES            # 1024
TOK_PER_CORE = SEQ_PER_CORE * CONTEXT      # 32768
GROUP_TOK = 512                            # tokens per group (16 seqs)
N_GROUPS = TOK_PER_CORE // GROUP_TOK       # 64


def _build_consts(wq, wk, wv, w1, b1, w2, b2):
    """Host-side constant prep. Matmul operands in bf16, rest fp32."""
    import ml_dtypes
    f32 = np.float32
    bf16 = ml_dtypes.bfloat16
    # A_h = wq_h @ wk_h^T * D^-0.5   [H, C, C]
    a_all = np.einsum("hcd,hed->hce", wq.astype(np.float64),
                      wk.astype(np.float64)) * (HEAD ** -0.5)
    # Wv concat over heads: [C, H*D]
    wv_c = np.ascontiguousarray(wv.transpose(1, 0, 2).reshape(NUM_EMB, NUM_EMB))
    w1_c = np.ascontiguousarray(w1)                        # [128, 512]
    w2_c = np.ascontiguousarray(w2.reshape(4, 128, NUM_EMB))  # [j, p, c]
    ident = np.eye(128)
    # mask via PE accumulation: Sp += maskI^T @ maskR
    #   maskI [s', (b4 s)] = I32 tiled 4x;  maskR [s', (k h t)] = -1e9*(s'>t)
    i32 = np.eye(32)
    maskI = np.tile(i32, (1, 4))                           # [32, 128]
    s_idx = np.arange(CONTEXT)[:, None]
    t_idx = np.arange(CONTEXT)[None, :]
    m0 = np.where(s_idx > t_idx, -1e9, 0.0)                # [s', t]
    maskR = np.tile(m0, (1, 4 * NUM_HEADS))                # [32, 512] (k,h tiled)
    return dict(
        a_all=np.ascontiguousarray(a_all).astype(bf16),
        wv_c=wv_c.astype(bf16), w1_c=w1_c.astype(bf16), w2_c=w2_c.astype(bf16),
        ident=np.ascontiguousarray(ident).astype(bf16),
        identf=np.ascontiguousarray(ident).astype(f32),
        maskI=np.ascontiguousarray(maskI).astype(bf16),
        maskR=np.ascontiguousarray(maskR).astype(bf16),
        b1_c=np.ascontiguousarray(b1).astype(f32).reshape(1, 4 * NUM_EMB),
        b2_c=np.ascontiguousarray(b2).astype(f32).reshape(1, NUM_EMB),
    )


def emit(ctx, tc, aps, n_groups):
    """Emit the per-core program.

    aps: dict of bass.AP handles keyed: xin, out, a_all, wv_c, w1_c, w2_c,
         maskb, ident, b1_c, b2_c
    """
    import concourse.bass as bass
    from concourse import mybir

    nc = tc.nc
    f32 = mybir.dt.float32
    bf16 = mybir.dt.bfloat16
    AF = mybir.ActivationFunctionType

    use_b1 = aps.get("use_b1", False)
    use_b2 = aps.get("use_b2", False)

    consts = ctx.enter_context(tc.tile_pool(name="consts", bufs=1))
    sb = ctx.enter_context(tc.tile_pool(name="sb", bufs=int(os.environ.get("SB_BUFS", "6"))))
    # PSUM budget, 8 banks: xt{XTp,Vp}(1) p{PT,Sp,ZTp}(3) opy{Op}(2) h{Hp,Yp}(2)
    ps = ctx.enter_context(tc.tile_pool(name="ps", bufs=1, space="PSUM"))
    ps2 = ctx.enter_context(tc.tile_pool(name="ps2", bufs=2, space="PSUM"))
    ps3 = ctx.enter_context(tc.tile_pool(name="ps3", bufs=3, space="PSUM"))

    # ---- load constants to SBUF (bf16 matmul operands) ----
    A_sb = consts.tile([128, NUM_HEADS, 128], bf16)             # [c, h, c']
    nc.sync.dma_start(out=A_sb, in_=aps["a_all"].rearrange("h c e -> c h e"))
    Wv_sb = consts.tile([128, 128], bf16)                       # [c, (h d)]
    nc.sync.dma_start(out=Wv_sb, in_=aps["wv_c"])
    W1_sb = consts.tile([128, 512], bf16)                       # [c, 4c]
    nc.sync.dma_start(out=W1_sb, in_=aps["w1_c"])
    W2_sb = consts.tile([128, 4, 128], bf16)                    # [p, j, c]
    nc.sync.dma_start(out=W2_sb, in_=aps["w2_c"].rearrange("j p c -> p j c"))
    MkI_sb = consts.tile([32, 128], bf16)                       # [s', (b4 s)]
    nc.sync.dma_start(out=MkI_sb, in_=aps["maskI"])
    MkR_sb = consts.tile([32, 512], bf16)                       # [s', (k h t)]
    nc.sync.dma_start(out=MkR_sb, in_=aps["maskR"])
    I_sb = consts.tile([128, 128], bf16)
    nc.sync.dma_start(out=I_sb, in_=aps["ident"])
    if use_b1 or use_b2:
        ones_sb = consts.tile([1, 512], f32)
        nc.vector.memset(ones_sb, 1.0)
    if use_b1:
        B1_sb = consts.tile([1, 512], f32)
        nc.sync.dma_start(out=B1_sb, in_=aps["b1_c"])
    if use_b2:
        B2_sb = consts.tile([1, 128], f32)
        nc.sync.dma_start(out=B2_sb, in_=aps["b2_c"])

    # persistent block-diagonal att buffers (zeros off-diag survive reuse)
    EBs = []
    for i in range(2):
        EBi = consts.tile([128, 4, NUM_HEADS, 128], bf16, name=f"EB{i}")
        nc.vector.memset(EBi.rearrange("p k h t -> p (k h t)"), 0.0)
        EBs.append(EBi)

    xin, out = aps["xin"], aps["out"]
    repeats = aps.get("repeats", 1)

    def front(g):
        """Group front-end: x load, X^T, Q/K/V projections."""
        rows = slice(g * GROUP_TOK, (g + 1) * GROUP_TOK)
        # ---- load x group, casting f32->bf16 in the DMA (SWDGE) ----
        Xn = sb.tile([128, 4, 128], bf16, tag="Xn")
        nc.gpsimd.dma_start(out=Xn, in_=xin[rows, :].rearrange("(k p) c -> p k c", p=128))

        # ---- X^T via PE (bf16 matmul vs identity: out = Xn.T @ I) ----
        XTp = ps.tile([128, 4, 128], f32, tag="xt")
        for k in range(4):
            nc.tensor.matmul(XTp[:, k, :], lhsT=Xn[:, k, :], rhs=I_sb,
                             start=True, stop=True, skip_group_check=True)
        XT = sb.tile([128, 4, 128], bf16, tag="XT")             # [c, k, tok]
        nc.vector.tensor_copy(XT, XTp)

        # ---- P^T_h = A_h^T @ X^T : [c', (k tok)] per head ----
        # (128-deep contraction; scores then need only col-group packing,
        # row+col 32x32 packing dies with an INTERNAL error on this stack)
        XTflat = XT.rearrange("c k t -> c (k t)")
        PT = sb.tile([128, NUM_HEADS, 4, 128], bf16, tag="PT")  # [c', h, k, tok]
        for h in range(NUM_HEADS):
            Pp = ps3.tile([128, 512], f32, tag="p")
            nc.tensor.matmul(Pp, lhsT=A_sb[:, h, :], rhs=XTflat, start=True, stop=True)
            dst = PT[:, h, :, :].rearrange("c k t -> c (k t)")
            if h % 2 == 0:
                nc.vector.tensor_copy(dst, Pp)
            else:
                nc.scalar.copy(dst, Pp)

        # ---- V = X @ Wv (+ones col): V33 [(b4 s), k, h, d|1] bf16 ----
        # (reuses the xt bank: XTp is consumed by the XT copy before V runs)
        Vp = ps.tile([128, 4, 128], f32, tag="xt")
        for k in range(4):
            nc.tensor.matmul(Vp[:, k, :], lhsT=XT[:, k, :], rhs=Wv_sb,
                             start=True, stop=True, skip_group_check=True)
        V33 = sb.tile([128, 4, NUM_HEADS, HEAD + 1], bf16, tag="V33")
        nc.vector.memset(V33[:, :, :, HEAD:HEAD + 1], 1.0)
        nc.vector.tensor_copy(V33[:, :, :, 0:HEAD],
                              Vp.rearrange("p k (h d) -> p k h d", h=NUM_HEADS))
        return dict(Xn=Xn, XT=XT, PT=PT, V33=V33)

    def back(g, ft):
        """Group back-end: scores through FFN and the output store."""
        rows = slice(g * GROUP_TOK, (g + 1) * GROUP_TOK)
        Xn, XT, PT, V33 = ft["Xn"], ft["XT"], ft["PT"], ft["V33"]

        # ---- attention mid-section, pipelined in two k-halves ----
        # Each half: mask+scores into its own PSUM bank -> exp -> spread ->
        # phase-2 MMs -> recip/normalize -> Z^T accumulation for its two k's.
        E = sb.tile([128, 4, NUM_HEADS, HEAD], bf16, tag="E")   # [(b4 s), k, h, t]
        EB = EBs[g % 2]
        R = sb.tile([128, 2, 2, NUM_HEADS], f32, tag="R")
        Tn = sb.tile([128, 4, NUM_HEADS, HEAD], bf16, tag="Tn")
        Tnf = Tn.rearrange("p k h d -> p k (h d)")
        ZTp = ps3.tile([128, 4, 128], f32, tag="p")
        for half in range(2):
            # scores^T block-diag: S^T[key, (h q)] = X A^T X^T, 128-deep
            Sp = ps3.tile([128, 2, NUM_HEADS, 32], f32, tag="p")
            nc.tensor.matmul(Sp.rearrange("p k h t -> p (k h t)"), lhsT=MkI_sb,
                             rhs=MkR_sb[:, 256 * half:256 * half + 256],
                             start=True, stop=False, skip_group_check=True)
            for k2 in range(2):
                k = 2 * half + k2
                for b4 in range(4):
                    ts = slice(32 * b4, 32 * b4 + 32)
                    nc.tensor.matmul(
                        Sp[ts, k2, :, :].rearrange("p h t -> p (h t)"),
                        lhsT=XT[:, k, ts],
                        rhs=PT[:, :, k, ts],
                        start=False,
                        stop=(k2 == 1 and b4 == 3),
                        skip_group_check=True,
                        tile_position=(0, 32 * b4))
            nc.scalar.activation(
                E[:, 2 * half:2 * half + 2, :, :].rearrange("p k h t -> p (k h t)"),
                Sp.rearrange("p k h t -> p (k h t)"), AF.Exp)
            for k2 in range(2):
                k = 2 * half + k2
                for b4 in range(4):
                    pr = slice(32 * b4, 32 * b4 + 32)
                    dst = EB[pr, k, :, 32 * b4:32 * b4 + 32]
                    if b4 == 1 and k2 == 0:
                        nc.vector.tensor_copy(dst, E[pr, k, :, :])
                    else:
                        nc.gpsimd.tensor_copy(dst, E[pr, k, :, :])
            Opb = ps2.tile([128, 512], f32, tag="opy")
            Op = Opb.rearrange("p (a h d) -> p a h d", a=2, h=NUM_HEADS)
            for k2 in range(2):
                k = 2 * half + k2
                for h in range(NUM_HEADS):
                    nc.tensor.matmul(
                        Op[:, k2, h, 0:HEAD + 1],
                        lhsT=EB[:, k, h, :],
                        rhs=V33[:, k, h, :],
                        start=True, stop=True, skip_group_check=True)
            nc.vector.reciprocal(
                R[:, half, :, :].rearrange("p b h -> p (b h)"),
                Op[:, :, :, HEAD:HEAD + 1].rearrange("p a h x -> p (a h x)"))
            nc.vector.tensor_mul(
                Tn[:, 2 * half:2 * half + 2, :, :], Op[:, :, :, 0:HEAD],
                R[:, half, :, :].unsqueeze(3).to_broadcast(
                    [128, 2, NUM_HEADS, HEAD]))
            for k2 in range(2):
                k = 2 * half + k2
                # Z^T = X^T + Tn^T via accumulating bf16 PE transpose
                nc.tensor.matmul(ZTp[:, k, :], lhsT=Xn[:, k, :], rhs=I_sb,
                                 start=True, stop=False, skip_group_check=True)
                nc.tensor.matmul(ZTp[:, k, :], lhsT=Tnf[:, k, :], rhs=I_sb,
                                 start=False, stop=True, skip_group_check=True)
        ZT = sb.tile([128, 4, 128], bf16, tag="ZT")
        nc.scalar.copy(ZT, ZTp)
        ZTflat = ZT.rearrange("c k t -> c (k t)")

        # ---- FFN1: H^T chunks [4c_j, (k tok)], relu-cast to bf16 ----
        Hs = sb.tile([128, 4, 512], bf16, tag="Hs")
        for j in range(4):
            Hp = ps2.tile([128, 512], f32, tag="h")
            if use_b1:
                nc.tensor.matmul(Hp, lhsT=B1_sb[:, 128 * j:128 * j + 128],
                                 rhs=ones_sb, start=True, stop=False,
                                 skip_group_check=True)
            nc.tensor.matmul(Hp, lhsT=W1_sb[:, 128 * j:128 * j + 128], rhs=ZTflat,
                             start=not use_b1, stop=True, skip_group_check=True)
            nc.scalar.activation(Hs[:, j, :], Hp, AF.Relu)

        # ---- FFN2 + residual (Z re-materialized on PE from ZT) ----
        Yo = sb.tile([128, 4, 128], f32, tag="Yo")
        Yp = ps2.tile([128, 4, 128], f32, tag="h")
        for k in range(4):
            if use_b2:
                nc.tensor.matmul(Yp[:, k, :], lhsT=ones_sb[:, 0:128], rhs=B2_sb,
                                 start=True, stop=False, skip_group_check=True)
            nc.tensor.matmul(Yp[:, k, :], lhsT=ZT[:, k, :], rhs=I_sb,
                             start=not use_b2, stop=False, skip_group_check=True)
            for j in range(4):
                nc.tensor.matmul(Yp[:, k, :], lhsT=Hs[:, j, 128 * k:128 * k + 128],
                                 rhs=W2_sb[:, j, :],
                                 start=False, stop=(j == 3),
                                 skip_group_check=True)
        nc.vector.tensor_copy(Yo, Yp)

        nc.sync.dma_start(out=out[rows, :].rearrange("(k p) c -> p k c", p=128),
                          in_=Yo)

    def all_groups():
        for g in range(n_groups):
            back(g, front(g))

    if repeats > 1:
        with tc.For_i(0, repeats, 1):
            all_groups()
    else:
        all_groups()


def build_program(n_groups, use_b1=False, use_b2=False, repeats=1):
    """Build Bass program; returns (nc, input_names)."""
    from contextlib import ExitStack

    import concourse.bass as bass
    import concourse.tile as tile
    from concourse import bacc, mybir

    f32 = mybir.dt.float32
    bf16 = mybir.dt.bfloat16
    nc = bacc.Bacc(trn_type="TRN2")
    tok = n_groups * GROUP_TOK
    aps = {
        "xin": nc.dram_tensor("xin", [tok, 128], f32, kind="ExternalInput")[:, :],
        "a_all": nc.dram_tensor("a_all", [NUM_HEADS, 128, 128], bf16, kind="ExternalInput")[:, :, :],
        "wv_c": nc.dram_tensor("wv_c", [128, 128], bf16, kind="ExternalInput")[:, :],
        "w1_c": nc.dram_tensor("w1_c", [128, 512], bf16, kind="ExternalInput")[:, :],
        "w2_c": nc.dram_tensor("w2_c", [4, 128, 128], bf16, kind="ExternalInput")[:, :, :],
        "maskI": nc.dram_tensor("maskI", [32, 128], bf16, kind="ExternalInput")[:, :],
        "maskR": nc.dram_tensor("maskR", [32, 512], bf16, kind="ExternalInput")[:, :],
        "ident": nc.dram_tensor("ident", [128, 128], bf16, kind="ExternalInput")[:, :],
        "identf": nc.dram_tensor("identf", [128, 128], f32, kind="ExternalInput")[:, :],
        "out": nc.dram_tensor("out", [tok, 128], f32, kind="ExternalOutput")[:, :],
        "use_b1": use_b1, "use_b2": use_b2, "repeats": repeats,
    }
    if use_b1:
        aps["b1_c"] = nc.dram_tensor("b1_c", [1, 512], f32, kind="ExternalInput")[:, :]
    if use_b2:
        aps["b2_c"] = nc.dram_tensor("b2_c", [1, 128], f32, kind="ExternalInput")[:, :]

    with ExitStack() as ctx:
        tc = ctx.enter_context(tile.TileContext(nc))
        emit(ctx, tc, aps, n_groups)
    nc.compile()
    return nc


_LAST_RESULTS = None  # BassKernelResults from the most recent kernel() call


def kernel(x, wq, wk, wv, w1, b1, w2, b2):
    """Full-input entry point: shards x over 8 cores, runs on HW, gathers."""
    global _LAST_RESULTS
    from concourse.bass_utils import run_bass_kernel_spmd

    in_maps, use_b1, use_b2 = _make_in_maps(x, wq, wk, wv, w1, b1, w2, b2)
    nc = build_program(N_GROUPS, use_b1, use_b2)
    res = run_bass_kernel_spmd(nc, in_maps, list(range(N_CORES)))
    _LAST_RESULTS = res
    out = np.concatenate([res.results[i]["out"] for i in range(N_CORES)], axis=0)
    return out.reshape(BATCH, CONTEXT, NUM_EMB).astype(np.float32)


def _make_in_maps(x, wq, wk, wv, w1, b1, w2, b2):
    consts = _build_consts(np.asarray(wq, np.float32), np.asarray(wk, np.float32),
                           np.asarray(wv, np.float32), np.asarray(w1, np.float32),
                           np.asarray(b1, np.float32), np.asarray(w2, np.float32),
                           np.asarray(b2, np.float32))
    use_b1 = bool(np.any(consts["b1_c"]))
    use_b2 = bool(np.any(consts["b2_c"]))
    const_map = {
        "a_all": consts["a_all"],
        "wv_c": consts["wv_c"], "w1_c": consts["w1_c"],
        "w2_c": consts["w2_c"], "maskI": consts["maskI"], "maskR": consts["maskR"],
        "ident": consts["ident"], "identf": consts["identf"],
    }
    if use_b1:
        const_map["b1_c"] = consts["b1_c"]
    if use_b2:
        const_map["b2_c"] = consts["b2_c"]
    shards = np.asarray(x, np.float32).reshape(N_CORES, TOK_PER_CORE, NUM_EMB)
    in_maps = [dict(xin=np.ascontiguousarray(shards[i]), **const_map)
               for i in range(N_CORES)]
    return in_maps, use_b1, use_b2


def bench_exec_time(np_inputs, r_hi=2001, reps=5):
    """Device-time estimate via repeat-loop slope: (t(r_hi) - t(1)) / (r_hi-1).

    Transfer/dispatch costs are identical for both programs and cancel.
    Runs are interleaved in (1, r_hi) pairs to cancel machine-load drift;
    the first pair (compile/cache warm-up) is discarded and the minimum
    paired slope is reported. Returns ns per single pass over the full
    workload.
    """
    import time

    from concourse.bass_utils import run_bass_kernel_spmd

    in_maps, use_b1, use_b2 = _make_in_maps(**np_inputs)
    progs = {r: build_program(N_GROUPS, use_b1, use_b2, repeats=r)
             for r in (1, r_hi)}

    pairs = []
    for i in range(reps):
        ts = {}
        for r in (1, r_hi):
            t0 = time.perf_counter()
            run_bass_kernel_spmd(progs[r], in_maps, list(range(N_CORES)))
            ts[r] = time.perf_counter() - t0
        pairs.append((ts[1], ts[r_hi]))
    slopes = [(b - a) / (r_hi - 1) * 1e9 for a, b in pairs[1:]]
    good = [s for s in slopes if s > 0] or slopes
    ns = min(good)
    print("bench pairs (ms): " +
          " ".join(f"({a*1e3:.0f},{b*1e3:.0f})" for a, b in pairs))
    print("bench slopes (ns/pass): " + " ".join(f"{s:.0f}" for s in slopes))
    return ns

